# revision 3
# baseline (speedup 1.0000x reference)
"""Causal self-attention (B=2, T=4096, D=512, H=8) on 8 Trainium2 NeuronCores.

Sharding: data parallel on batch (2 groups of 4 cores), tensor parallel on
heads (2 heads per core).  Fully interleaved schedule to keep the PE busy:
for each 512-row t-tile, a core
  1. projects q/k/v for its 2 heads (pre-transposed x so the contraction dim
     lands on partitions; q pre-scaled by 1/sqrt(dh) on host),
  2. runs causal attention chunks in the transposed layout: S^T[j,i] on PE
     (bf16), exp on ACT, causal mask on DVE, PV accumulate on PE with a
     ones-column in V yielding row-sums,
  3. normalizes via DVE reciprocal + PE broadcast (no ACT table swaps),
  4. projects its 128 attention dims into all 512 output dims and fires a
     4-core ReduceScatter over output-dim quarters for that t-tile; the 8
     small collectives pipeline behind compute, so only the last one tails.
Each core ends with y^T [128 out dims, 4096] for its batch; host reassembles.
"""

import os

import numpy as np

B, T, D = 2, 4096, 512
H = 8
DH = D // H  # 64
N_CORES = 8
TT = 512  # i-tile (query rows per tile)
JC = 128  # j-chunk (kv rows per chunk)
N_IT = T // TT  # 8
N_JC = T // JC  # 32
CC = 128  # contraction chunk
N_CC = D // CC  # 4
OUT_LAG = 3  # i-tiles between RS trigger and readback

LAST_EXEC_NS = None
_CACHE = {}


def _build_program():
    from contextlib import ExitStack

    import concourse.mybir as mybir
    import concourse.tile as tile
    from concourse import bacc
    from concourse.masks import make_identity

    fp32 = mybir.dt.float32
    bf16 = mybir.dt.bfloat16
    Exp = mybir.ActivationFunctionType.Exp

    nc = bacc.Bacc("TRN2", target_bir_lowering=False, debug=False,
                   num_devices=N_CORES)

    # ---- I/O -----------------------------------------------------------
    xT_d = nc.dram_tensor("xTt", [N_IT, 128, N_CC, TT], bf16,
                          kind="ExternalInput")
    wq_d = nc.dram_tensor("wq", [128, N_CC, 128], bf16, kind="ExternalInput")
    wk_d = nc.dram_tensor("wk", [128, N_CC, 128], bf16, kind="ExternalInput")
    wv_d = nc.dram_tensor("wv", [128, N_CC, 128], bf16, kind="ExternalInput")
    bq_d = nc.dram_tensor("bq", [128, 1], fp32, kind="ExternalInput")
    bk_d = nc.dram_tensor("bk", [128, 1], fp32, kind="ExternalInput")
    bv_d = nc.dram_tensor("bv", [128, 1], fp32, kind="ExternalInput")
    msk_d = nc.dram_tensor("msk", [128, 2 * JC], bf16, kind="ExternalInput")
    blk2_d = nc.dram_tensor("blk2", [2, 128], bf16, kind="ExternalInput")
    wp_d = nc.dram_tensor("wp", [128, D], bf16, kind="ExternalInput")
    bp_d = nc.dram_tensor("bp", [128, 1], fp32, kind="ExternalInput")
    yT_d = nc.dram_tensor("yT", [128, T], fp32, kind="ExternalOutput")

    with tile.TileContext(nc) as tc:
        with (
            tc.tile_pool(name="psum_mm", bufs=2, space="PSUM") as psum_mm,
            tc.tile_pool(name="psum_o", bufs=3, space="PSUM") as psum_o,
            tc.tile_pool(name="psum_bc", bufs=1, space="PSUM") as psum_bc,
            tc.tile_pool(name="ptiles", bufs=4) as ptiles,
            tc.tile_pool(name="small", bufs=2) as small,
            tc.tile_pool(name="ytiles", bufs=3) as ytiles,
            tc.tile_pool(name="dram", bufs=1, space="DRAM") as dram,
            ExitStack() as singles,
        ):
            def T_(shape, name, dt=bf16):
                t, free = tc.tile(shape, dt, name=name)
                singles.callback(free)
                return t

            # ---- persistent SBUF tensors -------------------------------
            xT_sb = T_([128, N_CC, T], "xT_sb")
            wq_sb = T_([128, N_CC, 128], "wq_sb")
            wk_sb = T_([128, N_CC, 128], "wk_sb")
            wv_sb = T_([128, N_CC, 128], "wv_sb")
            bq_sb = T_([128, 1], "bq_sb", fp32)
            bk_sb = T_([128, 1], "bk_sb", fp32)
            bv_sb = T_([128, 1], "bv_sb", fp32)
            msk_sb = T_([128, 2, JC], "msk_sb")
            wp_sb = T_([128, D], "wp_sb")
            bp_sb = T_([128, 1], "bp_sb", fp32)
            qT_sb = T_([128, T], "qT_sb")
            kT_sb = T_([128, T], "kT_sb")
            # v^T first; each tile's slice is consumed by the transposes
            # before the same slice is reused as the attention output attn^T
            vT_sb = T_([128, T], "vT_sb")
            attnT_sb = vT_sb
            # V in natural layout [t-chunk, head, DH+1]; col 64 = ones
            V_sb = T_([128, N_JC, 2, DH + 1], "V_sb")
            ident = T_([128, 128], "ident")
            blk2 = T_([2, 128], "blk2")

            make_identity(nc, ident[:])
            nc.vector.memset(V_sb[:, :, :, DH], 1.0)

            # ---- load inputs: small weights first, then x tiles --------
            for w_sb, w_d in ((wk_sb, wk_d), (wv_sb, wv_d), (wq_sb, wq_d)):
                nc.sync.dma_start(w_sb[:], w_d.ap())
            for b_sb, b_d in ((bk_sb, bk_d), (bv_sb, bv_d), (bq_sb, bq_d)):
                nc.sync.dma_start(b_sb[:], b_d.ap())
            nc.sync.dma_start(msk_sb[:], msk_d.ap())
            nc.sync.dma_start(blk2[:], blk2_d.ap())
            nc.sync.dma_start(wp_sb[:], wp_d.ap())
            nc.sync.dma_start(bp_sb[:], bp_d.ap())
            for tt in range(N_IT):
                nc.sync.dma_start(
                    xT_sb[:, :, tt * TT:(tt + 1) * TT], xT_d.ap()[tt])

            rs_in = [dram.tile([N_CC, 128, TT], bf16, name=f"rs_in{i}",
                               tag=f"rsi{i}") for i in range(N_IT)]
            rs_out = [dram.tile([128, TT], bf16, name=f"rs_out{i}",
                                tag=f"rso{i}") for i in range(N_IT)]

            def emit_qkv(tt):
                sl = slice(tt * TT, (tt + 1) * TT)
                for w_sb, b_sb, dst in (
                    (wk_sb, bk_sb, kT_sb),
                    (wv_sb, bv_sb, vT_sb),
                    (wq_sb, bq_sb, qT_sb),
                ):
                    mm_ps = psum_mm.tile([128, 2, TT], fp32, tag="mm")
                    for ci in range(N_CC):
                        nc.tensor.matmul(
                            mm_ps[:, 0, :], w_sb[:, ci, :], xT_sb[:, ci, sl],
                            start=(ci == 0), stop=(ci == N_CC - 1))
                    nc.vector.tensor_scalar_add(
                        dst[:, sl], mm_ps[:, 0, :], b_sb[:])
                # transpose this tile's v^T into natural layout
                for k in range(4):
                    jc = 4 * tt + k
                    tp_ps = psum_mm.tile([128, 2, TT], bf16, tag="mm")
                    nc.tensor.transpose(
                        tp_ps[:, 0, 0:JC], vT_sb[:, jc * JC:(jc + 1) * JC],
                        ident[:])
                    for h in range(2):
                        nc.vector.tensor_copy(
                            V_sb[:, jc, h, 0:DH],
                            tp_ps[:, 0, h * DH:(h + 1) * DH])

            def emit_out(it):
                isl = slice(it * TT, (it + 1) * TT)
                yo_sb = ytiles.tile([128, TT], bf16, tag="yo")
                nc.sync.dma_start(yo_sb[:], rs_out[it][:])
                yb_sb = ytiles.tile([128, TT], fp32, tag="yb")
                nc.gpsimd.tensor_scalar_add(yb_sb[:], yo_sb[:], bp_sb[:])
                nc.sync.dma_start(yT_d.ap()[:, isl], yb_sb[:])

            # ---- main interleaved loop ---------------------------------
            for it in range(N_IT):
                isl = slice(it * TT, (it + 1) * TT)
                if it == 0:
                    emit_qkv(0)
                o_ps = [psum_o.tile([DH + 1, TT], fp32, tag="o",
                                    name=f"o_ps{h}") for h in range(2)]
                njc = 4 * (it + 1)
                for jc in range(njc):
                    d = jc - 4 * it  # >= 0 on diagonal chunks
                    lo = max(d, 0) * JC  # first valid i column
                    s_pair = psum_mm.tile([128, 2, TT], fp32, tag="mm")
                    for h in range(2):
                        hsl = slice(h * DH, (h + 1) * DH)
                        nc.tensor.matmul(
                            s_pair[:, h, lo:TT],
                            kT_sb[hsl, jc * JC:(jc + 1) * JC],
                            qT_sb[hsl, it * TT + lo:(it + 1) * TT],
                            start=True, stop=True, skip_group_check=True)
                    p_pair = ptiles.tile([128, 2, TT], bf16, tag="p")
                    nc.scalar.activation(p_pair[:, :, lo:TT],
                                         s_pair[:, :, lo:TT], Exp)
                    if d >= 0:  # diagonal chunk: causal mask, both heads
                        nc.vector.tensor_mul(
                            p_pair[:, :, lo:lo + JC],
                            p_pair[:, :, lo:lo + JC], msk_sb[:])
                    for h in range(2):
                        nc.tensor.matmul(
                            o_ps[h][:, lo:TT], V_sb[:, jc, h, :],
                            p_pair[:, h, lo:TT],
                            start=(jc == 0), stop=(jc == njc - 1),
                            skip_group_check=True)

                # queue next tile's projections so the PE stays fed while
                # the normalization chain drains
                if it + 1 < N_IT:
                    emit_qkv(it + 1)

                # ---- normalize: attnT = o / rowsum (DVE + PE broadcast) -
                for h in range(2):
                    hsl = slice(h * DH, (h + 1) * DH)
                    recf = small.tile([1, TT], fp32, tag=f"recf{h}")
                    nc.vector.reciprocal(recf[:], o_ps[h][DH:DH + 1, :])
                    rec_b = small.tile([1, TT], bf16, tag=f"rec{h}")
                    nc.vector.tensor_copy(rec_b[:], recf[:])
                    bc_ps = psum_bc.tile([DH, TT], fp32, tag="bc")
                    nc.tensor.matmul(bc_ps[:], blk2[0:1, 0:DH], rec_b[:],
                                     start=True, stop=True)
                    bc_sb = small.tile([DH, TT], bf16, tag=f"bcs{h}")
                    nc.vector.tensor_copy(bc_sb[:], bc_ps[:])
                    nc.vector.tensor_mul(
                        attnT_sb[hsl, isl], o_ps[h][0:DH, :], bc_sb[:])

                # ---- partial output projection + per-tile ReduceScatter -
                for oc in range(N_CC):
                    y_ps = psum_mm.tile([128, 2, TT], fp32, tag="mm")
                    nc.tensor.matmul(
                        y_ps[:, 0, :], wp_sb[:, oc * 128:(oc + 1) * 128],
                        attnT_sb[:, isl], start=True, stop=True)
                    y_sb = ytiles.tile([128, TT], bf16, tag="y")
                    nc.vector.tensor_copy(y_sb[:], y_ps[:, 0, :])
                    nc.sync.dma_start(rs_in[it][oc], y_sb[:])
                nc.gpsimd.collective_compute(
                    "ReduceScatter", mybir.AluOpType.add,
                    replica_groups=[[0, 1, 2, 3], [4, 5, 6, 7]],
                    ins=[rs_in[it][:].opt()], outs=[rs_out[it][:].opt()])
                if it >= OUT_LAG:
                    emit_out(it - OUT_LAG)
            for it in range(N_IT - OUT_LAG, N_IT):
                emit_out(it)

    nc.compile()
    return nc


def _prep_inputs(x, w_qkv, b_qkv, w_proj, b_proj):
    import ml_dtypes

    bf16 = ml_dtypes.bfloat16
    # [128, 2*JC] causal mask for diagonal chunks, tiled for both heads:
    # mask[jrow, col] = 1 iff col >= jrow
    m1 = (np.arange(JC)[None, :] >= np.arange(128)[:, None]).astype(bf16)
    masks = np.tile(m1, (1, 2))
    blk2 = np.kron(np.eye(2), np.ones((1, DH))).astype(bf16)

    def pct(w):  # [D, n] -> [128, N_CC, n] with D = cc*128 + p
        n = w.shape[1]
        return np.ascontiguousarray(
            w.reshape(N_CC, 128, n).transpose(1, 0, 2))

    in_maps = []
    for c in range(N_CORES):
        b, hp = divmod(c, 4)
        col = hp * 2 * DH  # first column of this core's 2 heads
        xT = np.ascontiguousarray(x[b].T).astype(bf16)  # [D, T]
        # [N_IT, 128, N_CC, TT]
        xTt = np.ascontiguousarray(
            xT.reshape(N_CC, 128, N_IT, TT).transpose(2, 1, 0, 3))
        in_maps.append({
            "xTt": xTt,
            "wq": pct((w_qkv[:, col:col + 128]
                       * np.float32(0.125)).astype(bf16)),
            "wk": pct(w_qkv[:, D + col:D + col + 128].astype(bf16)),
            "wv": pct(w_qkv[:, 2 * D + col:2 * D + col + 128].astype(bf16)),
            "bq": (b_qkv[col:col + 128] * np.float32(0.125))
            .reshape(128, 1).copy(),
            "bk": b_qkv[D + col:D + col + 128].reshape(128, 1).copy(),
            "bv": b_qkv[2 * D + col:2 * D + col + 128].reshape(128, 1).copy(),
            "msk": masks,
            "blk2": blk2,
            "wp": np.ascontiguousarray(w_proj[col:col + 128, :]).astype(bf16),
            "bp": b_proj[hp * 128:(hp + 1) * 128].reshape(128, 1).copy(),
        })
    return in_maps


def kernel(x, w_qkv, b_qkv, w_proj, b_proj):
    global LAST_EXEC_NS
    from concourse.bass_utils import run_bass_kernel_spmd

    x = np.asarray(x, dtype=np.float32)
    w_qkv = np.asarray(w_qkv, dtype=np.float32)
    b_qkv = np.asarray(b_qkv, dtype=np.float32)
    w_proj = np.asarray(w_proj, dtype=np.float32)
    b_proj = np.asarray(b_proj, dtype=np.float32)

    if "nc" not in _CACHE:
        _CACHE["nc"] = _build_program()
    nc = _CACHE["nc"]

    in_maps = _prep_inputs(x, w_qkv, b_qkv, w_proj, b_proj)

    trace = bool(os.environ.get("BASS_KERNEL_TRACE"))
    kwargs = {}
    if trace:
        kwargs = {"trace": True,
                  "tmpdir": os.environ.get("BASS_KERNEL_TRACE_DIR") or None}
    res = run_bass_kernel_spmd(nc, in_maps, list(range(N_CORES)), **kwargs)
    LAST_EXEC_NS = res.exec_time_ns
    if trace:
        _CACHE["last_results"] = res

    # core c (group rank r = c%4) holds y^T for output dims [128r, 128r+128)
    # over its batch's full T
    out = np.empty((B, T, D), dtype=np.float32)
    for c in range(N_CORES):
        b, r = divmod(c, 4)
        yT = res.results[c]["yT"]
        out[b, :, r * 128:(r + 1) * 128] = yT.T
    return out
